# revision 34
# baseline (speedup 1.0000x reference)
"""MoE (8 experts, top-2) Bass kernel for 8 trn2 NeuronCores.

Strategy: data-parallel over tokens. Each core gets T/8 = 2048 tokens and all
expert weights (bf16). On device, per core:
  phase 1: router logits (fp32 matmul) -> top-2 via max8 -> combine weights via
           sigmoid -> per-expert positions via triangular-matmul prefix sums ->
           indirect-DMA scatter of (w, token_id) records into a position-
           ordered dispatch table; also stages x in bf16.
  phase 2: per expert: read its dispatch slice, dma_gather(transpose=True) the
           routed token rows directly into [H-chunk, c] matmul layout, dense
           FFN gelu(x@Wfc+bfc)@Wproj+bproj in bf16 with fp32 accumulate, scale
           rows by combine weight, dma_scatter_add the rows into the output by
           token id.

Host does only slicing/concat (and dtype staging of the weights to bf16).
"""

import math
import os
import sys

import numpy as np

for _p in ("/opt/trn_rl_repo", "/root/.axon_site/_ro/trn_rl_repo"):
    if os.path.isdir(_p) and _p not in sys.path:
        sys.path.insert(0, _p)

import ml_dtypes  # noqa: E402
import concourse.bass as bass  # noqa: E402
import concourse.mybir as mybir  # noqa: E402
import concourse.tile as tile  # noqa: E402
from concourse import bacc  # noqa: E402
from concourse.bass import IndirectOffsetOnAxis  # noqa: E402
from concourse.masks import make_identity, make_upper_triangular  # noqa: E402
from concourse import library_config  # noqa: E402

F32 = mybir.dt.float32
BF16 = mybir.dt.bfloat16
I32 = mybir.dt.int32
I16 = mybir.dt.int16
AF = mybir.ActivationFunctionType
ALU = mybir.AluOpType
AX = mybir.AxisListType

N_CORES = 8
P = 128
WARMUP_MM = 30


def _chunks(total, step):
    out = []
    off = 0
    while off < total:
        w = min(step, total - off)
        out.append((off, w))
        off += w
    return out


def build_moe(TLOC, H, F, E, CAPS, SLOP=128, zero_bias=False):
    """Build the per-core Bass program (SPMD: identical on all cores).

    CAPS: per-expert compute capacities (multiples of 64). Gather/scatter
    capacities are rounded up to 128 internally.
    """
    assert TLOC % P == 0 and H % P == 0 and F % P == 0 and E == 8
    CAPS = list(CAPS)
    assert len(CAPS) == E and all(c % 64 == 0 for c in CAPS)
    CAPRS = [((c + P - 1) // P) * P for c in CAPS]
    KH = H // P            # contraction chunks over H
    KF = F // P            # f-tiles (and stage-2 contraction chunks)
    NT = TLOC // P         # token tiles
    BASES = [sum(CAPRS[:e]) for e in range(E)]
    NPOS = sum(CAPRS) + SLOP
    NPOS = ((NPOS + P - 1) // P) * P
    HT = _chunks(H, 512)   # stage-2 output h-tiles

    nc = bacc.Bacc("TRN2", target_bir_lowering=False, debug=False,
                   enable_asserts=True, num_devices=N_CORES,
                   num_swdge_queues=4)

    xloc = nc.dram_tensor("xloc", [TLOC, H], F32, kind="ExternalInput")
    wr = nc.dram_tensor("wr", [H, E], F32, kind="ExternalInput")
    brr = nc.dram_tensor("brr", [1, E], F32, kind="ExternalInput")
    wfc = nc.dram_tensor("wfc", [E, H, F], BF16, kind="ExternalInput")
    bfc = nc.dram_tensor("bfc", [E, F], F32, kind="ExternalInput")
    wpj = nc.dram_tensor("wpj", [E, F, H], BF16, kind="ExternalInput")
    bpj = nc.dram_tensor("bpj", [E, H], BF16, kind="ExternalInput")
    # extra trash tile rows at the end absorb pad-slot scatter-adds
    out = nc.dram_tensor("out", [TLOC + P, H], F32, kind="ExternalOutput")

    wtbuf = nc.dram_tensor("wtbuf", [NPOS, 2], F32)   # (combine w, token id)
    xbf = nc.dram_tensor("xbf", [TLOC + P, H], BF16)  # x staged in bf16

    with tile.TileContext(nc) as tc:
        # ---------------- constants ----------------
        with tc.tile_pool(name="const", bufs=1) as cpool:
            id_f32 = cpool.tile([P, P], F32, tag="id_f32")
            make_identity(nc, id_f32)
            id_bf = cpool.tile([P, P], BF16, tag="id_bf")
            make_identity(nc, id_bf)
            u_incl = cpool.tile([P, P], F32, tag="u_incl")
            make_upper_triangular(nc, u_incl, val=1.0, diag=True)
            u_strict = cpool.tile([P, P], F32, tag="u_strict")
            make_upper_triangular(nc, u_strict, val=1.0, diag=False)
            ones_f = cpool.tile([1, P], F32, tag="ones_f")
            nc.gpsimd.memset(ones_f[:], 1.0)
            ones_col = cpool.tile([P, 1], F32, tag="ones_col")
            nc.gpsimd.memset(ones_col[:], 1.0)
            ones_bf = cpool.tile([1, P], BF16, tag="ones_bf")
            nc.gpsimd.memset(ones_bf[:], 1.0)
            zbias = cpool.tile([P, 1], F32, tag="zbias")
            nc.gpsimd.memset(zbias[:], 0.0)
            tid_base = cpool.tile([P, 1], F32, tag="tid_base")
            tid_base_i = cpool.tile([P, 1], I32, tag="tid_base_i")
            nc.gpsimd.iota(tid_base_i[:], pattern=[[0, 1]], base=0,
                           channel_multiplier=1)
            nc.vector.tensor_copy(out=tid_base[:], in_=tid_base_i[:])
            bases = cpool.tile([1, E], F32, tag="bases")
            for e in range(E):
                nc.gpsimd.memset(bases[:, e:e + 1], float(BASES[e] - 1))
            zw = cpool.tile([P, 2], F32, tag="zw")
            nc.gpsimd.memset(zw[:, 0:1], 0.0)
            nc.gpsimd.memset(zw[:, 1:2], float(TLOC))
            zbig = cpool.tile([P, H], F32, tag="zbig")
            nc.gpsimd.memset(zbig[:], 0.0)
            zbig_bf = cpool.tile([P, H], BF16, tag="zbig_bf")
            nc.gpsimd.memset(zbig_bf[:], 0.0)
            br_sb = cpool.tile([1, E], F32, tag="br_sb")
            nc.sync.dma_start(out=br_sb[:], in_=brr.ap()[:, :])

            # PE warmup: keep the HAM clock-gate open while inputs stream in
            wzero = cpool.tile([P, 512], BF16, tag="wzero")
            nc.gpsimd.memset(wzero[:], 0.0)
            with tc.tile_pool(name="wu", bufs=2, space="PSUM") as wup:
                for wi in range(WARMUP_MM):
                    wps = wup.tile([P, 512], F32, tag="wps")
                    nc.tensor.matmul(out=wps[:], lhsT=id_bf[:], rhs=wzero[:],
                                     start=True, stop=True)


            # init dispatch table (w=0, tid=trash) and zero out + xbf tail
            # (traced after warmup so router input loads win queue priority;
            # WAW deps still order these before the dispatch scatters)
            for a in range(NPOS // P):
                nc.sync.dma_start(out=wtbuf.ap()[a * P:(a + 1) * P, :],
                                  in_=zw[:])
            for i in range(NT + 1):
                nc.sync.dma_start(out=out.ap()[i * P:(i + 1) * P, :],
                                  in_=zbig[:])
            nc.sync.dma_start(out=xbf.ap()[TLOC:TLOC + P, :], in_=zbig_bf[:])

            # ================= PHASE 1: router + dispatch =================
            with tc.tile_pool(name="ph1", bufs=4) as ph1, \
                 tc.tile_pool(name="ph1k", bufs=1) as ph1k, \
                 tc.tile_pool(name="ps_tr", bufs=2, space="PSUM") as ps_tr, \
                 tc.tile_pool(name="ps_pos", bufs=2, space="PSUM") as ps_pos, \
                 tc.tile_pool(name="ps_sm", bufs=1, space="PSUM") as ps_sm:
                wr_sb = ph1k.tile([P, KH * E], F32, tag="wr_sb")
                for k in range(KH):
                    nc.sync.dma_start(
                        out=wr_sb[:, k * E:(k + 1) * E],
                        in_=wr.ap()[k * P:(k + 1) * P, :])

                eq1 = {}
                eq2 = {}
                msk = {}
                wvs = {}
                off_i = {}
                cnt_sb = ph1k.tile([NT, E], F32, tag="cnt_sb")
                GROUPS = [range(0, NT // 2), range(NT // 2, NT)]

                def router_tile(i):
                    x_t = ph1.tile([P, H], F32, tag="x_t", name="x_t")
                    nc.sync.dma_start(out=x_t[:],
                                      in_=xloc.ap()[i * P:(i + 1) * P, :])
                    xbf_t = ph1.tile([P, H], BF16, tag="xbf_t", name="xbf_t")
                    nc.vector.tensor_copy(out=xbf_t[:], in_=x_t[:])
                    nc.sync.dma_start(out=xbf.ap()[i * P:(i + 1) * P, :],
                                      in_=xbf_t[:])
                    lg_ps = ps_sm.tile([P, E], F32, tag="lg_ps", name="lg_ps")
                    for k in range(KH):
                        tp = ps_tr.tile([P, P], F32, tag="tp", name="tp")
                        nc.tensor.transpose(
                            out=tp[:], in_=x_t[:, k * P:(k + 1) * P],
                            identity=id_f32[:])
                        xT = ph1.tile([P, P], F32, tag="xT", name="xT")
                        if k % 2 == 0:
                            nc.vector.tensor_copy(out=xT[:], in_=tp[:])
                        else:
                            nc.scalar.copy(out=xT[:], in_=tp[:])
                        nc.tensor.matmul(
                            out=lg_ps[:], lhsT=xT[:],
                            rhs=wr_sb[:, k * E:(k + 1) * E],
                            start=(k == 0), stop=False)
                    nc.tensor.matmul(out=lg_ps[:], lhsT=ones_f[:, :P],
                                     rhs=br_sb[:], start=False, stop=True)
                    lg = ph1.tile([P, E], F32, tag="lg", name="lg")
                    nc.vector.tensor_copy(out=lg[:], in_=lg_ps[:])
                    m8 = ph1.tile([P, 8], F32, tag="m8", name="m8")
                    nc.vector.max(out=m8[:], in_=lg[:])
                    e1 = ph1k.tile([P, E], F32, tag=f"eq1_{i}", name=f"eq1_{i}")
                    nc.vector.tensor_tensor(
                        out=e1[:], in0=lg[:],
                        in1=m8[:, 0:1].to_broadcast([P, E]), op=ALU.is_equal)
                    e2 = ph1k.tile([P, E], F32, tag=f"eq2_{i}", name=f"eq2_{i}")
                    nc.vector.tensor_tensor(
                        out=e2[:], in0=lg[:],
                        in1=m8[:, 1:2].to_broadcast([P, E]), op=ALU.is_equal)
                    mk = ph1k.tile([P, E], F32, tag=f"msk_{i}", name=f"msk_{i}")
                    nc.vector.tensor_add(out=mk[:], in0=e1[:], in1=e2[:])
                    dt_ = ph1.tile([P, 2], F32, tag="dt_", name="dt_")
                    nc.vector.tensor_sub(out=dt_[:, 0:1], in0=m8[:, 0:1],
                                         in1=m8[:, 1:2])
                    nc.vector.tensor_sub(out=dt_[:, 1:2], in0=m8[:, 1:2],
                                         in1=m8[:, 0:1])
                    wv = ph1k.tile([P, 2], F32, tag=f"wv_{i}", name=f"wv_{i}")
                    nc.scalar.activation(out=wv[:], in_=dt_[:], func=AF.Sigmoid,
                                         bias=zbias[:])
                    eq1[i] = e1
                    eq2[i] = e2
                    msk[i] = mk
                    wvs[i] = wv
                    cnt_row = ps_sm.tile([1, E], F32, tag="cnt_row",
                                         name="cnt_row")
                    nc.tensor.matmul(out=cnt_row[:], lhsT=ones_col[:, 0:1],
                                     rhs=mk[:], start=True, stop=True)
                    cnt_row_sb = ph1.tile([1, E], F32, tag="cnt_row_sb",
                                          name="cnt_row_sb")
                    nc.vector.tensor_copy(out=cnt_row_sb[:], in_=cnt_row[:])
                    nc.sync.dma_start(out=cnt_sb[i:i + 1, :], in_=cnt_row_sb[:])

                def offsets(g, rows):
                    n = rows.stop
                    off_ps = ps_sm.tile([n, E], F32, tag=f"off_ps{g}",
                                        name=f"off_ps{g}")
                    nc.tensor.matmul(out=off_ps[:], lhsT=u_strict[0:n, 0:n],
                                     rhs=cnt_sb[0:n, :], start=True, stop=False)
                    nc.tensor.matmul(out=off_ps[:], lhsT=ones_f[:, :n],
                                     rhs=bases[:], start=False, stop=True)
                    off_sb = ph1.tile([n, E], F32, tag=f"off_sb{g}",
                                      name=f"off_sb{g}")
                    nc.vector.tensor_copy(out=off_sb[:], in_=off_ps[:])
                    for i in rows:
                        oi = ph1k.tile([1, E], F32, tag=f"off_{i}",
                                       name=f"off_{i}")
                        nc.sync.dma_start(out=oi[:], in_=off_sb[i:i + 1, :])
                        off_i[i] = oi

                def disp_tile(i, d1l, d2l, wp1l, wp2l):
                    pos_ps = ps_pos.tile([P, E], F32, tag="pos_ps",
                                         name="pos_ps")
                    nc.tensor.matmul(out=pos_ps[:], lhsT=u_incl[:],
                                     rhs=msk[i][:], start=True, stop=False)
                    nc.tensor.matmul(out=pos_ps[:], lhsT=ones_f[:, :P],
                                     rhs=off_i[i][0:1, :], start=False,
                                     stop=True)
                    pos = ph1.tile([P, E], F32, tag="pos", name="pos")
                    nc.vector.tensor_copy(out=pos[:], in_=pos_ps[:])
                    tmp = ph1.tile([P, E], F32, tag="tmp", name="tmp")
                    d1f = ph1.tile([P, 1], F32, tag="d1f", name="d1f")
                    d2f = ph1.tile([P, 1], F32, tag="d2f", name="d2f")
                    nc.vector.tensor_mul(out=tmp[:], in0=eq1[i][:], in1=pos[:])
                    nc.vector.reduce_sum(out=d1f[:], in_=tmp[:], axis=AX.X)
                    nc.vector.tensor_mul(out=tmp[:], in0=eq2[i][:], in1=pos[:])
                    nc.vector.reduce_sum(out=d2f[:], in_=tmp[:], axis=AX.X)
                    d1 = ph1k.tile([P, 1], I32, tag=f"d1_{i}", name=f"d1_{i}")
                    d2 = ph1k.tile([P, 1], I32, tag=f"d2_{i}", name=f"d2_{i}")
                    nc.vector.tensor_copy(out=d1[:], in_=d1f[:])
                    nc.vector.tensor_copy(out=d2[:], in_=d2f[:])
                    wp1 = ph1k.tile([P, 2], F32, tag=f"wp1_{i}",
                                    name=f"wp1_{i}")
                    wp2 = ph1k.tile([P, 2], F32, tag=f"wp2_{i}",
                                    name=f"wp2_{i}")
                    nc.vector.tensor_copy(out=wp1[:, 0:1], in_=wvs[i][:, 0:1])
                    nc.vector.tensor_copy(out=wp2[:, 0:1], in_=wvs[i][:, 1:2])
                    nc.vector.tensor_scalar_add(wp1[:, 1:2], tid_base[:],
                                                float(i * P))
                    nc.vector.tensor_scalar_add(wp2[:, 1:2], tid_base[:],
                                                float(i * P))
                    d1l.append(d1)
                    d2l.append(d2)
                    wp1l.append(wp1)
                    wp2l.append(wp2)

                disp_sem = nc.alloc_semaphore("disp_dma")
                sem_count = 0
                for g, rows in enumerate(GROUPS):
                    for i in rows:
                        router_tile(i)
                    offsets(g, rows)
                    d1l = []
                    d2l = []
                    wp1l = []
                    wp2l = []
                    for i in rows:
                        disp_tile(i, d1l, d2l, wp1l, wp2l)
                    # batch this group's scatters back-to-back (rows disjoint
                    # by construction -> no ordering between them needed)
                    with tc.tile_critical():
                        for j in range(len(d1l)):
                            nc.gpsimd.indirect_dma_start(
                                out=wtbuf.ap(),
                                out_offset=IndirectOffsetOnAxis(
                                    ap=d1l[j][:, 0:1], axis=0),
                                in_=wp1l[j][:, :], in_offset=None).then_inc(
                                    disp_sem, 16)
                            nc.gpsimd.indirect_dma_start(
                                out=wtbuf.ap(),
                                out_offset=IndirectOffsetOnAxis(
                                    ap=d2l[j][:, 0:1], axis=0),
                                in_=wp2l[j][:, :], in_offset=None).then_inc(
                                    disp_sem, 16)
                            sem_count += 32
                        nc.gpsimd.wait_ge(disp_sem, sem_count)

            # dma_gather / dma_scatter_add ucode lives in the mlp library
            nc.gpsimd.load_library(library_config.mlp)

            # ================= PHASE 2: expert FFN + combine ==============
            with tc.tile_pool(name="wcache", bufs=2) as wc, \
                 tc.tile_pool(name="ph2", bufs=3) as ph2, \
                 tc.tile_pool(name="wpp", bufs=6) as wpp, \
                 tc.tile_pool(name="xet", bufs=2) as xetp, \
                 tc.tile_pool(name="h1t", bufs=1) as h1tp, \
                 tc.tile_pool(name="ysb", bufs=2) as ysbp, \
                 tc.tile_pool(name="wtk", bufs=2) as wtkp, \
                 tc.tile_pool(name="ps_s1", bufs=2, space="PSUM") as ps_s1, \
                 tc.tile_pool(name="ps_y", bufs=1, space="PSUM") as ps_y:
                EORDER = sorted(range(E), key=lambda e: -CAPS[e])
                for e in EORDER:
                    CAP = CAPS[e]          # compute capacity (mult of 64)
                    CAPR = CAPRS[e]        # gather/scatter capacity (mult 128)
                    BASE = BASES[e]
                    NSUB = CAPR // P
                    SUBS = _chunks(CAP, P)
                    MACROS = _chunks(CAP, 512)
                    wfc_k = []
                    for k in range(KH):
                        wk = wc.tile([P, F], BF16, tag=f"wfc{k}", name=f"wfc{k}")
                        nc.sync.dma_start(
                            out=wk[:], in_=wfc.ap()[e, k * P:(k + 1) * P, :])
                        wfc_k.append(wk)
                    bfc_sb = wc.tile([P, KF], F32, tag="bfc_sb")
                    nc.sync.dma_start(
                        out=bfc_sb[:],
                        in_=bfc.ap()[e:e + 1, :].rearrange(
                            "o (a p) -> p (o a)", p=P))
                    bpj_sb = wc.tile([1, H], BF16, tag="bpj_sb")
                    nc.sync.dma_start(out=bpj_sb[:], in_=bpj.ap()[e:e + 1, :])

                    # dispatch metadata: wrapped token-id indices,
                    # replicated into each gpsimd core's 16-partition group
                    tidw_f = ph2.tile([P, CAPR // 16], F32, tag="tidw_f")
                    for g in range(P // 16):
                        nc.sync.dma_start(
                            out=tidw_f[16 * g:16 * (g + 1), :],
                            in_=wtbuf.ap()[BASE:BASE + CAPR, 1:2].rearrange(
                                "(s p) o -> p (s o)", p=16))
                    idx = wtkp.tile([P, CAPR // 16], I16, tag="idx")
                    nc.vector.tensor_copy(out=idx[:], in_=tidw_f[:])
                    w_t = []
                    for ci, (cs, cw) in enumerate(SUBS):
                        wt = wtkp.tile([P, 1], F32, tag=f"wt{ci}",
                                       name=f"wt{ci}")
                        nc.sync.dma_start(
                            out=wt[0:cw, :],
                            in_=wtbuf.ap()[BASE + cs:BASE + cs + cw, 0:1])
                        w_t.append(wt)

                    # gather x rows transposed: xet3d[p, k, c] = x[tid[c], k*128+p]
                    xet3d = xetp.tile([P, KH * CAPR], BF16, tag="xet3d")
                    nc.gpsimd.dma_gather(
                        out_ap=xet3d[:, :].rearrange("p (k c) -> p k c", k=KH),
                        in_ap=xbf.ap()[:, :],
                        idxs_ap=idx[:, :], num_idxs=CAPR, num_idxs_reg=CAPR,
                        elem_size=H, transpose=True,
                        queue_num=1 + (e % 2))

                    h1t = [h1tp.tile([P, CAP], BF16, tag=f"h1t{ft}",
                                     name=f"h1t{ft}") for ft in range(KF)]
                    for (ms, mw) in MACROS:
                        for ft in range(KF):
                            ps1 = ps_s1.tile([P, 512], F32, tag="ps1")
                            for k in range(KH):
                                nc.tensor.matmul(
                                    out=ps1[:, 0:mw],
                                    lhsT=wfc_k[k][:, ft * P:(ft + 1) * P],
                                    rhs=xet3d[:, k * CAPR + ms:k * CAPR + ms + mw],
                                    start=(k == 0), stop=(k == KH - 1))
                            nc.scalar.activation(
                                out=h1t[ft][:, ms:ms + mw], in_=ps1[:, 0:mw],
                                func=AF.Gelu_apprx_tanh,
                                bias=(zbias[:] if zero_bias
                                      else bfc_sb[:, ft:ft + 1]))

                    ysb = ysbp.tile([P, NSUB * H], F32, tag="ysb")
                    if CAP < CAPR:
                        nc.vector.memset(
                            ysb[CAP % P:P, (CAP // P) * H:(CAP // P + 1) * H],
                            0.0)
                    for (hs, hw) in HT:
                        psy = [ps_y.tile([P, hw], F32, tag=f"psy{ci}",
                                         name=f"psy{ci}")
                               for ci in range(len(SUBS))]
                        for k in range(KF):
                            wp = wpp.tile([P, 512], BF16, tag="wp")
                            nc.sync.dma_start(
                                out=wp[:, 0:hw],
                                in_=wpj.ap()[e, k * P:(k + 1) * P, hs:hs + hw])
                            for ci, (cs, cw) in enumerate(SUBS):
                                nc.tensor.matmul(
                                    out=psy[ci][0:cw, 0:hw],
                                    lhsT=h1t[k][:, cs:cs + cw],
                                    rhs=wp[:, 0:hw],
                                    start=(k == 0),
                                    stop=(zero_bias and k == KF - 1))
                        for ci, (cs, cw) in enumerate(SUBS):
                            if not zero_bias:
                                nc.tensor.matmul(
                                    out=psy[ci][0:cw, 0:hw],
                                    lhsT=ones_bf[:, 0:cw],
                                    rhs=bpj_sb[:, hs:hs + hw],
                                    start=False, stop=True)
                            nc.vector.tensor_scalar_mul(
                                ysb[0:cw, ci * H + hs:ci * H + hs + hw],
                                psy[ci][0:cw, 0:hw],
                                w_t[ci][0:cw, 0:1])
                    nc.gpsimd.dma_scatter_add(
                        out_ap=out.ap()[:, :],
                        in_ap=ysb[:, :].rearrange("p (n h) -> p n h", n=NSUB),
                        idxs_ap=idx[:, :], num_idxs=CAPR, num_idxs_reg=CAPR,
                        elem_size=H, queue_num=3)

    nc.compile()
    return nc


# ---------------------------------------------------------------------------
_BUILD_CACHE = {}
_LAST_IN_MAPS = None


def _get_built(TLOC, H, F, E, CAPS, zero_bias=False):
    key = (TLOC, H, F, E, tuple(CAPS), zero_bias)
    if key not in _BUILD_CACHE:
        _BUILD_CACHE[key] = build_moe(TLOC, H, F, E, tuple(CAPS),
                                      zero_bias=zero_bias)
    return _BUILD_CACHE[key]


def kernel(hidden_states, Wr, br, Wfc, bfc, Wproj, bproj):
    from concourse.bass_utils import run_bass_kernel_spmd

    hs = np.ascontiguousarray(np.asarray(hidden_states, dtype=np.float32))
    Wr = np.ascontiguousarray(np.asarray(Wr, dtype=np.float32))
    br = np.ascontiguousarray(np.asarray(br, dtype=np.float32))
    Wfc = np.asarray(Wfc, dtype=np.float32)
    bfc = np.ascontiguousarray(np.asarray(bfc, dtype=np.float32))
    Wproj = np.asarray(Wproj, dtype=np.float32)
    bproj = np.asarray(bproj, dtype=np.float32)

    B, S, H = hs.shape
    E, H2, F = Wfc.shape
    assert H2 == H
    T = B * S
    assert T % N_CORES == 0
    TLOC = T // N_CORES
    x = hs.reshape(T, H)

    # host-side routing peek ONLY to pick the compile-time capacities
    logits = x @ Wr + br[None, :]
    top2 = np.argpartition(-logits, 2, axis=1)[:, :2]
    maxcnt = np.zeros(E, dtype=np.int64)
    for c in range(N_CORES):
        sl = top2[c * TLOC:(c + 1) * TLOC]
        cnts = np.bincount(sl.ravel(), minlength=E)
        maxcnt = np.maximum(maxcnt, cnts)
    CAPS = tuple(int(max(128, math.ceil((m + 8) / 64.0) * 64))
                 for m in maxcnt)

    zero_bias = bool(np.all(bfc == 0.0) and np.all(bproj == 0.0))
    nc = _get_built(TLOC, H, F, E, CAPS, zero_bias)

    wfc_bf = np.ascontiguousarray(Wfc.astype(ml_dtypes.bfloat16))
    wpj_bf = np.ascontiguousarray(Wproj.astype(ml_dtypes.bfloat16))
    bpj_bf = np.ascontiguousarray(bproj.astype(ml_dtypes.bfloat16))
    br_row = np.ascontiguousarray(br.reshape(1, E))

    in_maps = []
    for c in range(N_CORES):
        in_maps.append({
            "xloc": np.ascontiguousarray(x[c * TLOC:(c + 1) * TLOC]),
            "wr": Wr,
            "brr": br_row,
            "wfc": wfc_bf,
            "bfc": bfc,
            "wpj": wpj_bf,
            "bpj": bpj_bf,
        })

    global _LAST_IN_MAPS
    _LAST_IN_MAPS = in_maps

    res = run_bass_kernel_spmd(nc, in_maps, core_ids=list(range(N_CORES)))
    outs = [res.results[c]["out"][:TLOC] for c in range(N_CORES)]
    return np.concatenate(outs, axis=0).reshape(B, S, H).astype(np.float32)


# revision 35
# speedup vs baseline: 1.0022x; 1.0022x over previous
"""MoE (8 experts, top-2) Bass kernel for 8 trn2 NeuronCores.

Strategy: data-parallel over tokens. Each core gets T/8 = 2048 tokens and all
expert weights (bf16). On device, per core:
  phase 1: router logits (fp32 matmul) -> top-2 via max8 -> combine weights via
           sigmoid -> per-expert positions via triangular-matmul prefix sums ->
           indirect-DMA scatter of (w, token_id) records into a position-
           ordered dispatch table; also stages x in bf16.
  phase 2: per expert: read its dispatch slice, dma_gather(transpose=True) the
           routed token rows directly into [H-chunk, c] matmul layout, dense
           FFN gelu(x@Wfc+bfc)@Wproj+bproj in bf16 with fp32 accumulate, scale
           rows by combine weight, dma_scatter_add the rows into the output by
           token id.

Host does only slicing/concat (and dtype staging of the weights to bf16).
"""

import math
import os
import sys

import numpy as np

for _p in ("/opt/trn_rl_repo", "/root/.axon_site/_ro/trn_rl_repo"):
    if os.path.isdir(_p) and _p not in sys.path:
        sys.path.insert(0, _p)

import ml_dtypes  # noqa: E402
import concourse.bass as bass  # noqa: E402
import concourse.mybir as mybir  # noqa: E402
import concourse.tile as tile  # noqa: E402
from concourse import bacc  # noqa: E402
from concourse.bass import IndirectOffsetOnAxis  # noqa: E402
from concourse.masks import make_identity, make_upper_triangular  # noqa: E402
from concourse import library_config  # noqa: E402

F32 = mybir.dt.float32
BF16 = mybir.dt.bfloat16
I32 = mybir.dt.int32
I16 = mybir.dt.int16
AF = mybir.ActivationFunctionType
ALU = mybir.AluOpType
AX = mybir.AxisListType

N_CORES = 8
P = 128
WARMUP_MM = 30


def _chunks(total, step):
    out = []
    off = 0
    while off < total:
        w = min(step, total - off)
        out.append((off, w))
        off += w
    return out


def build_moe(TLOC, H, F, E, CAPS, SLOP=128, zero_bias=False):
    """Build the per-core Bass program (SPMD: identical on all cores).

    CAPS: per-expert compute capacities (multiples of 64). Gather/scatter
    capacities are rounded up to 128 internally.
    """
    assert TLOC % P == 0 and H % P == 0 and F % P == 0 and E == 8
    CAPS = list(CAPS)
    assert len(CAPS) == E and all(c % 64 == 0 for c in CAPS)
    CAPRS = [((c + P - 1) // P) * P for c in CAPS]
    KH = H // P            # contraction chunks over H
    KF = F // P            # f-tiles (and stage-2 contraction chunks)
    NT = TLOC // P         # token tiles
    BASES = [sum(CAPRS[:e]) for e in range(E)]
    NPOS = sum(CAPRS) + SLOP
    NPOS = ((NPOS + P - 1) // P) * P
    HT = _chunks(H, 512)   # stage-2 output h-tiles

    nc = bacc.Bacc("TRN2", target_bir_lowering=False, debug=False,
                   enable_asserts=True, num_devices=N_CORES)

    xloc = nc.dram_tensor("xloc", [TLOC, H], F32, kind="ExternalInput")
    wr = nc.dram_tensor("wr", [H, E], F32, kind="ExternalInput")
    brr = nc.dram_tensor("brr", [1, E], F32, kind="ExternalInput")
    wfc = nc.dram_tensor("wfc", [E, H, F], BF16, kind="ExternalInput")
    bfc = nc.dram_tensor("bfc", [E, F], F32, kind="ExternalInput")
    wpj = nc.dram_tensor("wpj", [E, F, H], BF16, kind="ExternalInput")
    bpj = nc.dram_tensor("bpj", [E, H], BF16, kind="ExternalInput")
    # extra trash tile rows at the end absorb pad-slot scatter-adds
    out = nc.dram_tensor("out", [TLOC + P, H], F32, kind="ExternalOutput")

    wtbuf = nc.dram_tensor("wtbuf", [NPOS, 2], F32)   # (combine w, token id)
    xbf = nc.dram_tensor("xbf", [TLOC + P, H], BF16)  # x staged in bf16

    with tile.TileContext(nc) as tc:
        # ---------------- constants ----------------
        with tc.tile_pool(name="const", bufs=1) as cpool:
            id_f32 = cpool.tile([P, P], F32, tag="id_f32")
            make_identity(nc, id_f32)
            id_bf = cpool.tile([P, P], BF16, tag="id_bf")
            make_identity(nc, id_bf)
            u_incl = cpool.tile([P, P], F32, tag="u_incl")
            make_upper_triangular(nc, u_incl, val=1.0, diag=True)
            u_strict = cpool.tile([P, P], F32, tag="u_strict")
            make_upper_triangular(nc, u_strict, val=1.0, diag=False)
            ones_f = cpool.tile([1, P], F32, tag="ones_f")
            nc.gpsimd.memset(ones_f[:], 1.0)
            ones_col = cpool.tile([P, 1], F32, tag="ones_col")
            nc.gpsimd.memset(ones_col[:], 1.0)
            ones_bf = cpool.tile([1, P], BF16, tag="ones_bf")
            nc.gpsimd.memset(ones_bf[:], 1.0)
            zbias = cpool.tile([P, 1], F32, tag="zbias")
            nc.gpsimd.memset(zbias[:], 0.0)
            tid_base = cpool.tile([P, 1], F32, tag="tid_base")
            tid_base_i = cpool.tile([P, 1], I32, tag="tid_base_i")
            nc.gpsimd.iota(tid_base_i[:], pattern=[[0, 1]], base=0,
                           channel_multiplier=1)
            nc.vector.tensor_copy(out=tid_base[:], in_=tid_base_i[:])
            bases = cpool.tile([1, E], F32, tag="bases")
            for e in range(E):
                nc.gpsimd.memset(bases[:, e:e + 1], float(BASES[e] - 1))
            zw = cpool.tile([P, 2], F32, tag="zw")
            nc.gpsimd.memset(zw[:, 0:1], 0.0)
            nc.gpsimd.memset(zw[:, 1:2], float(TLOC))
            zbig = cpool.tile([P, H], F32, tag="zbig")
            nc.gpsimd.memset(zbig[:], 0.0)
            zbig_bf = cpool.tile([P, H], BF16, tag="zbig_bf")
            nc.gpsimd.memset(zbig_bf[:], 0.0)
            br_sb = cpool.tile([1, E], F32, tag="br_sb")
            nc.sync.dma_start(out=br_sb[:], in_=brr.ap()[:, :])

            # PE warmup: keep the HAM clock-gate open while inputs stream in
            wzero = cpool.tile([P, 512], BF16, tag="wzero")
            nc.gpsimd.memset(wzero[:], 0.0)
            with tc.tile_pool(name="wu", bufs=2, space="PSUM") as wup:
                for wi in range(WARMUP_MM):
                    wps = wup.tile([P, 512], F32, tag="wps")
                    nc.tensor.matmul(out=wps[:], lhsT=id_bf[:], rhs=wzero[:],
                                     start=True, stop=True)


            # init dispatch table (w=0, tid=trash) and zero out + xbf tail
            # (traced after warmup so router input loads win queue priority;
            # WAW deps still order these before the dispatch scatters)
            for a in range(NPOS // P):
                nc.sync.dma_start(out=wtbuf.ap()[a * P:(a + 1) * P, :],
                                  in_=zw[:])
            for i in range(NT + 1):
                nc.sync.dma_start(out=out.ap()[i * P:(i + 1) * P, :],
                                  in_=zbig[:])
            nc.sync.dma_start(out=xbf.ap()[TLOC:TLOC + P, :], in_=zbig_bf[:])

            # ================= PHASE 1: router + dispatch =================
            with tc.tile_pool(name="ph1", bufs=4) as ph1, \
                 tc.tile_pool(name="ph1k", bufs=1) as ph1k, \
                 tc.tile_pool(name="ps_tr", bufs=2, space="PSUM") as ps_tr, \
                 tc.tile_pool(name="ps_pos", bufs=2, space="PSUM") as ps_pos, \
                 tc.tile_pool(name="ps_sm", bufs=1, space="PSUM") as ps_sm:
                wr_sb = ph1k.tile([P, KH * E], F32, tag="wr_sb")
                for k in range(KH):
                    nc.sync.dma_start(
                        out=wr_sb[:, k * E:(k + 1) * E],
                        in_=wr.ap()[k * P:(k + 1) * P, :])

                eq1 = {}
                eq2 = {}
                msk = {}
                wvs = {}
                off_i = {}
                cnt_sb = ph1k.tile([NT, E], F32, tag="cnt_sb")
                GROUPS = [range(0, NT // 2), range(NT // 2, NT)]

                def router_tile(i):
                    x_t = ph1.tile([P, H], F32, tag="x_t", name="x_t")
                    nc.sync.dma_start(out=x_t[:],
                                      in_=xloc.ap()[i * P:(i + 1) * P, :])
                    xbf_t = ph1.tile([P, H], BF16, tag="xbf_t", name="xbf_t")
                    nc.vector.tensor_copy(out=xbf_t[:], in_=x_t[:])
                    nc.sync.dma_start(out=xbf.ap()[i * P:(i + 1) * P, :],
                                      in_=xbf_t[:])
                    lg_ps = ps_sm.tile([P, E], F32, tag="lg_ps", name="lg_ps")
                    for k in range(KH):
                        tp = ps_tr.tile([P, P], F32, tag="tp", name="tp")
                        nc.tensor.transpose(
                            out=tp[:], in_=x_t[:, k * P:(k + 1) * P],
                            identity=id_f32[:])
                        xT = ph1.tile([P, P], F32, tag="xT", name="xT")
                        if k % 2 == 0:
                            nc.vector.tensor_copy(out=xT[:], in_=tp[:])
                        else:
                            nc.scalar.copy(out=xT[:], in_=tp[:])
                        nc.tensor.matmul(
                            out=lg_ps[:], lhsT=xT[:],
                            rhs=wr_sb[:, k * E:(k + 1) * E],
                            start=(k == 0), stop=False)
                    nc.tensor.matmul(out=lg_ps[:], lhsT=ones_f[:, :P],
                                     rhs=br_sb[:], start=False, stop=True)
                    lg = ph1.tile([P, E], F32, tag="lg", name="lg")
                    nc.vector.tensor_copy(out=lg[:], in_=lg_ps[:])
                    m8 = ph1.tile([P, 8], F32, tag="m8", name="m8")
                    nc.vector.max(out=m8[:], in_=lg[:])
                    e1 = ph1k.tile([P, E], F32, tag=f"eq1_{i}", name=f"eq1_{i}")
                    nc.vector.tensor_tensor(
                        out=e1[:], in0=lg[:],
                        in1=m8[:, 0:1].to_broadcast([P, E]), op=ALU.is_equal)
                    e2 = ph1k.tile([P, E], F32, tag=f"eq2_{i}", name=f"eq2_{i}")
                    nc.vector.tensor_tensor(
                        out=e2[:], in0=lg[:],
                        in1=m8[:, 1:2].to_broadcast([P, E]), op=ALU.is_equal)
                    mk = ph1k.tile([P, E], F32, tag=f"msk_{i}", name=f"msk_{i}")
                    nc.vector.tensor_add(out=mk[:], in0=e1[:], in1=e2[:])
                    dt_ = ph1.tile([P, 2], F32, tag="dt_", name="dt_")
                    nc.vector.tensor_sub(out=dt_[:, 0:1], in0=m8[:, 0:1],
                                         in1=m8[:, 1:2])
                    nc.vector.tensor_sub(out=dt_[:, 1:2], in0=m8[:, 1:2],
                                         in1=m8[:, 0:1])
                    wv = ph1k.tile([P, 2], F32, tag=f"wv_{i}", name=f"wv_{i}")
                    nc.scalar.activation(out=wv[:], in_=dt_[:], func=AF.Sigmoid,
                                         bias=zbias[:])
                    eq1[i] = e1
                    eq2[i] = e2
                    msk[i] = mk
                    wvs[i] = wv
                    cnt_row = ps_sm.tile([1, E], F32, tag="cnt_row",
                                         name="cnt_row")
                    nc.tensor.matmul(out=cnt_row[:], lhsT=ones_col[:, 0:1],
                                     rhs=mk[:], start=True, stop=True)
                    cnt_row_sb = ph1.tile([1, E], F32, tag="cnt_row_sb",
                                          name="cnt_row_sb")
                    nc.vector.tensor_copy(out=cnt_row_sb[:], in_=cnt_row[:])
                    nc.sync.dma_start(out=cnt_sb[i:i + 1, :], in_=cnt_row_sb[:])

                def offsets(g, rows):
                    n = rows.stop
                    off_ps = ps_sm.tile([n, E], F32, tag=f"off_ps{g}",
                                        name=f"off_ps{g}")
                    nc.tensor.matmul(out=off_ps[:], lhsT=u_strict[0:n, 0:n],
                                     rhs=cnt_sb[0:n, :], start=True, stop=False)
                    nc.tensor.matmul(out=off_ps[:], lhsT=ones_f[:, :n],
                                     rhs=bases[:], start=False, stop=True)
                    off_sb = ph1.tile([n, E], F32, tag=f"off_sb{g}",
                                      name=f"off_sb{g}")
                    nc.vector.tensor_copy(out=off_sb[:], in_=off_ps[:])
                    for i in rows:
                        oi = ph1k.tile([1, E], F32, tag=f"off_{i}",
                                       name=f"off_{i}")
                        nc.sync.dma_start(out=oi[:], in_=off_sb[i:i + 1, :])
                        off_i[i] = oi

                def disp_tile(i, d1l, d2l, wp1l, wp2l):
                    pos_ps = ps_pos.tile([P, E], F32, tag="pos_ps",
                                         name="pos_ps")
                    nc.tensor.matmul(out=pos_ps[:], lhsT=u_incl[:],
                                     rhs=msk[i][:], start=True, stop=False)
                    nc.tensor.matmul(out=pos_ps[:], lhsT=ones_f[:, :P],
                                     rhs=off_i[i][0:1, :], start=False,
                                     stop=True)
                    pos = ph1.tile([P, E], F32, tag="pos", name="pos")
                    nc.vector.tensor_copy(out=pos[:], in_=pos_ps[:])
                    tmp = ph1.tile([P, E], F32, tag="tmp", name="tmp")
                    d1f = ph1.tile([P, 1], F32, tag="d1f", name="d1f")
                    d2f = ph1.tile([P, 1], F32, tag="d2f", name="d2f")
                    nc.vector.tensor_mul(out=tmp[:], in0=eq1[i][:], in1=pos[:])
                    nc.vector.reduce_sum(out=d1f[:], in_=tmp[:], axis=AX.X)
                    nc.vector.tensor_mul(out=tmp[:], in0=eq2[i][:], in1=pos[:])
                    nc.vector.reduce_sum(out=d2f[:], in_=tmp[:], axis=AX.X)
                    d1 = ph1k.tile([P, 1], I32, tag=f"d1_{i}", name=f"d1_{i}")
                    d2 = ph1k.tile([P, 1], I32, tag=f"d2_{i}", name=f"d2_{i}")
                    nc.vector.tensor_copy(out=d1[:], in_=d1f[:])
                    nc.vector.tensor_copy(out=d2[:], in_=d2f[:])
                    wp1 = ph1k.tile([P, 2], F32, tag=f"wp1_{i}",
                                    name=f"wp1_{i}")
                    wp2 = ph1k.tile([P, 2], F32, tag=f"wp2_{i}",
                                    name=f"wp2_{i}")
                    nc.vector.tensor_copy(out=wp1[:, 0:1], in_=wvs[i][:, 0:1])
                    nc.vector.tensor_copy(out=wp2[:, 0:1], in_=wvs[i][:, 1:2])
                    nc.vector.tensor_scalar_add(wp1[:, 1:2], tid_base[:],
                                                float(i * P))
                    nc.vector.tensor_scalar_add(wp2[:, 1:2], tid_base[:],
                                                float(i * P))
                    d1l.append(d1)
                    d2l.append(d2)
                    wp1l.append(wp1)
                    wp2l.append(wp2)

                disp_sem = nc.alloc_semaphore("disp_dma")
                sem_count = 0
                for g, rows in enumerate(GROUPS):
                    for i in rows:
                        router_tile(i)
                    offsets(g, rows)
                    d1l = []
                    d2l = []
                    wp1l = []
                    wp2l = []
                    for i in rows:
                        disp_tile(i, d1l, d2l, wp1l, wp2l)
                    # batch this group's scatters back-to-back (rows disjoint
                    # by construction -> no ordering between them needed)
                    with tc.tile_critical():
                        for j in range(len(d1l)):
                            nc.gpsimd.indirect_dma_start(
                                out=wtbuf.ap(),
                                out_offset=IndirectOffsetOnAxis(
                                    ap=d1l[j][:, 0:1], axis=0),
                                in_=wp1l[j][:, :], in_offset=None).then_inc(
                                    disp_sem, 16)
                            nc.gpsimd.indirect_dma_start(
                                out=wtbuf.ap(),
                                out_offset=IndirectOffsetOnAxis(
                                    ap=d2l[j][:, 0:1], axis=0),
                                in_=wp2l[j][:, :], in_offset=None).then_inc(
                                    disp_sem, 16)
                            sem_count += 32
                        nc.gpsimd.wait_ge(disp_sem, sem_count)

            # dma_gather / dma_scatter_add ucode lives in the mlp library
            nc.gpsimd.load_library(library_config.mlp)

            # ================= PHASE 2: expert FFN + combine ==============
            with tc.tile_pool(name="wcache", bufs=2) as wc, \
                 tc.tile_pool(name="ph2", bufs=3) as ph2, \
                 tc.tile_pool(name="wpp", bufs=6) as wpp, \
                 tc.tile_pool(name="xet", bufs=2) as xetp, \
                 tc.tile_pool(name="h1t", bufs=1) as h1tp, \
                 tc.tile_pool(name="ysb", bufs=2) as ysbp, \
                 tc.tile_pool(name="wtk", bufs=2) as wtkp, \
                 tc.tile_pool(name="ps_s1", bufs=2, space="PSUM") as ps_s1, \
                 tc.tile_pool(name="ps_y", bufs=1, space="PSUM") as ps_y:
                EORDER = sorted(range(E), key=lambda e: -CAPS[e])
                for e in EORDER:
                    CAP = CAPS[e]          # compute capacity (mult of 64)
                    CAPR = CAPRS[e]        # gather/scatter capacity (mult 128)
                    BASE = BASES[e]
                    NSUB = CAPR // P
                    SUBS = _chunks(CAP, P)
                    MACROS = _chunks(CAP, 512)
                    wfc_k = []
                    for k in range(KH):
                        wk = wc.tile([P, F], BF16, tag=f"wfc{k}", name=f"wfc{k}")
                        nc.sync.dma_start(
                            out=wk[:], in_=wfc.ap()[e, k * P:(k + 1) * P, :])
                        wfc_k.append(wk)
                    bfc_sb = wc.tile([P, KF], F32, tag="bfc_sb")
                    nc.sync.dma_start(
                        out=bfc_sb[:],
                        in_=bfc.ap()[e:e + 1, :].rearrange(
                            "o (a p) -> p (o a)", p=P))
                    bpj_sb = wc.tile([1, H], BF16, tag="bpj_sb")
                    nc.sync.dma_start(out=bpj_sb[:], in_=bpj.ap()[e:e + 1, :])

                    # dispatch metadata: wrapped token-id indices,
                    # replicated into each gpsimd core's 16-partition group
                    tidw_f = ph2.tile([P, CAPR // 16], F32, tag="tidw_f")
                    for g in range(P // 16):
                        nc.sync.dma_start(
                            out=tidw_f[16 * g:16 * (g + 1), :],
                            in_=wtbuf.ap()[BASE:BASE + CAPR, 1:2].rearrange(
                                "(s p) o -> p (s o)", p=16))
                    idx = wtkp.tile([P, CAPR // 16], I16, tag="idx")
                    nc.vector.tensor_copy(out=idx[:], in_=tidw_f[:])
                    w_t = []
                    for ci, (cs, cw) in enumerate(SUBS):
                        wt = wtkp.tile([P, 1], F32, tag=f"wt{ci}",
                                       name=f"wt{ci}")
                        nc.sync.dma_start(
                            out=wt[0:cw, :],
                            in_=wtbuf.ap()[BASE + cs:BASE + cs + cw, 0:1])
                        w_t.append(wt)

                    # gather x rows transposed: xet3d[p, k, c] = x[tid[c], k*128+p]
                    xet3d = xetp.tile([P, KH * CAPR], BF16, tag="xet3d")
                    nc.gpsimd.dma_gather(
                        out_ap=xet3d[:, :].rearrange("p (k c) -> p k c", k=KH),
                        in_ap=xbf.ap()[:, :],
                        idxs_ap=idx[:, :], num_idxs=CAPR, num_idxs_reg=CAPR,
                        elem_size=H, transpose=True)

                    h1t = [h1tp.tile([P, CAP], BF16, tag=f"h1t{ft}",
                                     name=f"h1t{ft}") for ft in range(KF)]
                    for (ms, mw) in MACROS:
                        for ft in range(KF):
                            ps1 = ps_s1.tile([P, 512], F32, tag="ps1")
                            for k in range(KH):
                                nc.tensor.matmul(
                                    out=ps1[:, 0:mw],
                                    lhsT=wfc_k[k][:, ft * P:(ft + 1) * P],
                                    rhs=xet3d[:, k * CAPR + ms:k * CAPR + ms + mw],
                                    start=(k == 0), stop=(k == KH - 1))
                            nc.scalar.activation(
                                out=h1t[ft][:, ms:ms + mw], in_=ps1[:, 0:mw],
                                func=AF.Gelu_apprx_tanh,
                                bias=(zbias[:] if zero_bias
                                      else bfc_sb[:, ft:ft + 1]))

                    ysb = ysbp.tile([P, NSUB * H], F32, tag="ysb")
                    if CAP < CAPR:
                        nc.vector.memset(
                            ysb[CAP % P:P, (CAP // P) * H:(CAP // P + 1) * H],
                            0.0)
                    for (hs, hw) in HT:
                        psy = [ps_y.tile([P, hw], F32, tag=f"psy{ci}",
                                         name=f"psy{ci}")
                               for ci in range(len(SUBS))]
                        for k in range(KF):
                            wp = wpp.tile([P, 512], BF16, tag="wp")
                            nc.sync.dma_start(
                                out=wp[:, 0:hw],
                                in_=wpj.ap()[e, k * P:(k + 1) * P, hs:hs + hw])
                            for ci, (cs, cw) in enumerate(SUBS):
                                nc.tensor.matmul(
                                    out=psy[ci][0:cw, 0:hw],
                                    lhsT=h1t[k][:, cs:cs + cw],
                                    rhs=wp[:, 0:hw],
                                    start=(k == 0),
                                    stop=(zero_bias and k == KF - 1))
                        for ci, (cs, cw) in enumerate(SUBS):
                            if not zero_bias:
                                nc.tensor.matmul(
                                    out=psy[ci][0:cw, 0:hw],
                                    lhsT=ones_bf[:, 0:cw],
                                    rhs=bpj_sb[:, hs:hs + hw],
                                    start=False, stop=True)
                            nc.vector.tensor_scalar_mul(
                                ysb[0:cw, ci * H + hs:ci * H + hs + hw],
                                psy[ci][0:cw, 0:hw],
                                w_t[ci][0:cw, 0:1])
                    nc.gpsimd.dma_scatter_add(
                        out_ap=out.ap()[:, :],
                        in_ap=ysb[:, :].rearrange("p (n h) -> p n h", n=NSUB),
                        idxs_ap=idx[:, :], num_idxs=CAPR, num_idxs_reg=CAPR,
                        elem_size=H)

    nc.compile()
    return nc


# ---------------------------------------------------------------------------
_BUILD_CACHE = {}
_LAST_IN_MAPS = None


def _get_built(TLOC, H, F, E, CAPS, zero_bias=False):
    key = (TLOC, H, F, E, tuple(CAPS), zero_bias)
    if key not in _BUILD_CACHE:
        _BUILD_CACHE[key] = build_moe(TLOC, H, F, E, tuple(CAPS),
                                      zero_bias=zero_bias)
    return _BUILD_CACHE[key]


def kernel(hidden_states, Wr, br, Wfc, bfc, Wproj, bproj):
    from concourse.bass_utils import run_bass_kernel_spmd

    hs = np.ascontiguousarray(np.asarray(hidden_states, dtype=np.float32))
    Wr = np.ascontiguousarray(np.asarray(Wr, dtype=np.float32))
    br = np.ascontiguousarray(np.asarray(br, dtype=np.float32))
    Wfc = np.asarray(Wfc, dtype=np.float32)
    bfc = np.ascontiguousarray(np.asarray(bfc, dtype=np.float32))
    Wproj = np.asarray(Wproj, dtype=np.float32)
    bproj = np.asarray(bproj, dtype=np.float32)

    B, S, H = hs.shape
    E, H2, F = Wfc.shape
    assert H2 == H
    T = B * S
    assert T % N_CORES == 0
    TLOC = T // N_CORES
    x = hs.reshape(T, H)

    # host-side routing peek ONLY to pick the compile-time capacities
    logits = x @ Wr + br[None, :]
    top2 = np.argpartition(-logits, 2, axis=1)[:, :2]
    maxcnt = np.zeros(E, dtype=np.int64)
    for c in range(N_CORES):
        sl = top2[c * TLOC:(c + 1) * TLOC]
        cnts = np.bincount(sl.ravel(), minlength=E)
        maxcnt = np.maximum(maxcnt, cnts)
    CAPS = tuple(int(max(128, math.ceil((m + 8) / 64.0) * 64))
                 for m in maxcnt)

    zero_bias = bool(np.all(bfc == 0.0) and np.all(bproj == 0.0))
    nc = _get_built(TLOC, H, F, E, CAPS, zero_bias)

    wfc_bf = np.ascontiguousarray(Wfc.astype(ml_dtypes.bfloat16))
    wpj_bf = np.ascontiguousarray(Wproj.astype(ml_dtypes.bfloat16))
    bpj_bf = np.ascontiguousarray(bproj.astype(ml_dtypes.bfloat16))
    br_row = np.ascontiguousarray(br.reshape(1, E))

    in_maps = []
    for c in range(N_CORES):
        in_maps.append({
            "xloc": np.ascontiguousarray(x[c * TLOC:(c + 1) * TLOC]),
            "wr": Wr,
            "brr": br_row,
            "wfc": wfc_bf,
            "bfc": bfc,
            "wpj": wpj_bf,
            "bpj": bpj_bf,
        })

    global _LAST_IN_MAPS
    _LAST_IN_MAPS = in_maps

    res = run_bass_kernel_spmd(nc, in_maps, core_ids=list(range(N_CORES)))
    outs = [res.results[c]["out"][:TLOC] for c in range(N_CORES)]
    return np.concatenate(outs, axis=0).reshape(B, S, H).astype(np.float32)
